# revision 8
# baseline (speedup 1.0000x reference)
"""Trainium2 Bass kernel v3 for nn_BaseGraphEncoder (gnn_message_passing).

Computation (per batch b):
    h0 = max_k x[:, idx[b,n,k]]            (gather + K-max, "local maxpool")
    h1 = h0 @ W1 + b1
    r  = relu(Wc1 @ h1 + bc1)              (conv1d k=1)
    h2 = max_k r[:, idx[b,n,k]]            (second local maxpool, same idx)
    h3 = h2 @ W2 + b2
    out = Wc2 @ h3 + bc2                   (conv1d k=1, 1024 out channels)

Sharding: data-parallel over batch B=16 across 8 NeuronCores (2 batches/core).

v3 = v1's HBM dma_gather parallelized over 4 SWDGE queues (the per-queue
SWDGE gather rate is ~16.6 ns/idx; 4 queues run ~3-4x faster in aggregate)
+ v2's bf16 compute pipeline:
  - gather elements are 256-B rows: 128 bf16 features (64 real + 64 zero
    pad), k-major index order so the K-max is a strided halving tree of
    DVE tensor_tensor(max) ops on the point-major gather output.
  - point-major -> feature-major via PE identity transposes (16 per layer).
  - conv1 is computed point-major (lhsT = h1 slices), so its output rw is
    directly in the wrapped layout [p, q*128+f] = r[q*128+p, f]; one
    contiguous 512-KiB DMA writes it to the DRAM table for the second
    gather.  The second gather's index VALUES are host-permuted
    (v -> (v%128)*16 + v//128) to match that row order - no on-device
    scatter needed.
  - all matmuls bf16 (PSUM fp32), biases preloaded into PSUM via K=1
    matmuls where they vary along the free dim; output fp32.

SWDGE queue-safety invariant (verified post-compile): Tile tracks SWDGE DMA
completion on 8 cumulative sem lanes assigned round-robin in final program
order; two same-lane gathers on different queues can complete out of order
and release a consumer early.  We emit NCHUNK (a multiple of 8) gathers per
layer-batch with queue = chunk % 4, so lane l always carries queue l%4.
_verify_gather_queue_alignment() raises if scheduling ever breaks this.
"""

import sys

if "/opt/trn_rl_repo" not in sys.path:
    sys.path.insert(0, "/opt/trn_rl_repo")

import numpy as np

import concourse.bacc as bacc
import concourse.bass as bass
import concourse.mybir as mybir
import concourse.tile as tile
from concourse._compat import get_trn_type
from concourse.bass_utils import run_bass_kernel_spmd

B, C, N, K = 16, 64, 2048, 16
NCORES = 8
BPC = B // NCORES  # batches per core
D1, D2, DOUT = 64, 128, 1024  # hidden dims
NIDX = N * K  # 32768 gather indices per batch per layer
NCHUNK = 16  # gather chunks per layer (queue = chunk % 4; overridable)
CH_IDX = NIDX // NCHUNK
CP = 128  # padded feature count (gather element = 128 bf16 = 256 B)
F32 = mybir.dt.float32
BF16 = mybir.dt.bfloat16
I16 = mybir.dt.int16

_compiled = None


def _verify_gather_queue_alignment(nc, nqueues):
    """Assert lane l (i%8 over Pool DMAs in final order) only sees one queue."""
    lane_q = {}
    i = 0
    for blk in nc.m.functions[0].blocks:
        for inst in blk.instructions:
            if type(inst).__name__ == "InstDMAGatherAnt":
                lane = i % 8
                q = inst.queue_num
                if lane in lane_q:
                    assert lane_q[lane] == q, (
                        f"SWDGE lane {lane} carries queues {lane_q[lane]} and "
                        f"{q}: cross-queue completion on a shared Tile DMASW "
                        f"lane races. Restructure gather emission."
                    )
                lane_q[lane] = q
                i += 1


def _build_nc(reps=1, gbufs=24, nqueues=4, nchunk=None):
    NCHUNK = nchunk or 16
    CH_IDX = NIDX // NCHUNK
    assert NCHUNK % nqueues == 0, "queue rotation must align with DMASW lanes"
    nc = bacc.Bacc(
        get_trn_type() or "TRN2",
        target_bir_lowering=False,
        debug=False,
        num_swdge_queues=nqueues,
    )

    xh_d = nc.dram_tensor("xh", [BPC, N, CP], BF16, kind="ExternalInput")
    idxa_d = nc.dram_tensor("idxa", [BPC, 128, NIDX // 16], I16, kind="ExternalInput")
    idxb_d = nc.dram_tensor("idxb", [BPC, 128, NIDX // 16], I16, kind="ExternalInput")
    w1_d = nc.dram_tensor("W1", [C, D1], BF16, kind="ExternalInput")
    wc1t_d = nc.dram_tensor("Wc1T", [D1, CP], BF16, kind="ExternalInput")
    w2_d = nc.dram_tensor("W2", [D1, D2], BF16, kind="ExternalInput")
    wc2t_d = nc.dram_tensor("Wc2T", [D2, DOUT], BF16, kind="ExternalInput")
    b1_d = nc.dram_tensor("b1", [D1, 1], F32, kind="ExternalInput")
    b2_d = nc.dram_tensor("b2", [D2, 1], F32, kind="ExternalInput")
    bc1r_d = nc.dram_tensor("bc1row4", [1, 512], BF16, kind="ExternalInput")
    bc2r_d = nc.dram_tensor("bc2row", [1, DOUT], BF16, kind="ExternalInput")
    id_d = nc.dram_tensor("ident", [128, 128], BF16, kind="ExternalInput")
    out_d = nc.dram_tensor("out", [BPC, DOUT, N], F32, kind="ExternalOutput")

    with tile.TileContext(nc) as tc:
        with (
            tc.tile_pool(name="consts", bufs=1) as consts,
            tc.tile_pool(name="idxpool", bufs=2) as idxpool,
            tc.tile_pool(name="gpool", bufs=gbufs) as gpool,
            tc.tile_pool(name="redpool", bufs=1) as redpool,
            tc.tile_pool(name="accpool", bufs=2) as accpool,
            tc.tile_pool(name="fmpool", bufs=2) as fmpool,
            tc.tile_pool(name="h1pool", bufs=2) as h1pool,
            tc.tile_pool(name="rwpool", bufs=2) as rwpool,
            tc.tile_pool(name="h3pool", bufs=2) as h3pool,
            tc.tile_pool(name="outpool", bufs=3) as outpool,
            tc.tile_pool(name="psm", bufs=2, space="PSUM") as psm,
            tc.tile_pool(name="pc1", bufs=2, space="PSUM") as pc1,
            tc.tile_pool(name="pst", bufs=2, space="PSUM") as pst,
            tc.tile_pool(name="pso", bufs=2, space="PSUM") as pso,
            tc.tile_pool(name="drampool", bufs=2, space="DRAM") as drampool,
        ):
            # ---- constants ----
            w1_sb = consts.tile([C, D1], BF16, tag="w1")
            wc1t_sb = consts.tile([D1, CP], BF16, tag="wc1t")
            w2_sb = consts.tile([D1, D2], BF16, tag="w2")
            wc2t_sb = consts.tile([D2, DOUT], BF16, tag="wc2t")
            b1_sb = consts.tile([D1, 1], F32, tag="b1")
            b2_sb = consts.tile([D2, 1], F32, tag="b2")
            bc1r_sb = consts.tile([1, 512], BF16, tag="bc1r")
            bc2r_sb = consts.tile([1, DOUT], BF16, tag="bc2r")
            id_sb = consts.tile([128, 128], BF16, tag="ident")
            ones_sb = consts.tile([1, 512], BF16, tag="ones")
            nc.sync.dma_start(w1_sb, w1_d[:])
            nc.sync.dma_start(wc1t_sb, wc1t_d[:])
            nc.sync.dma_start(w2_sb, w2_d[:])
            nc.sync.dma_start(wc2t_sb, wc2t_d[:])
            nc.sync.dma_start(b1_sb, b1_d[:])
            nc.sync.dma_start(b2_sb, b2_d[:])
            nc.sync.dma_start(bc1r_sb, bc1r_d[:])
            nc.sync.dma_start(bc2r_sb, bc2r_d[:])
            nc.sync.dma_start(id_sb, id_d[:])
            nc.vector.memset(ones_sb, 1.0)

            def gather_max_layer(src_dram_ap, idx_sb):
                """4-queue HBM dma_gather (k-major) + DVE halving-tree max.

                Returns point-major wrapped accP [128, 16*CP] bf16:
                accP[p, q*CP + f] = max-pooled feature f of point q*128+p.
                """
                accP = accpool.tile([128, (N // 128) * CP], BF16, tag="accP")
                for ch in range(NCHUNK):
                    g = gpool.tile([128, CH_IDX // 128, CP], BF16, tag="g")
                    nc.gpsimd.dma_gather(
                        g,
                        src_dram_ap,
                        idx_sb[:, ch * (CH_IDX // 16) : (ch + 1) * (CH_IDX // 16)],
                        CH_IDX,
                        CH_IDX,
                        CP,
                        single_packet=False,
                        queue_num=ch % nqueues,
                    )
                    gv = g.rearrange("p a c -> p (a c)")
                    # halving tree: [128, CH_IDX] -> [128, 2048] (cols bf16)
                    cur, width = gv, CH_IDX
                    while width > 2 * 2048:
                        width //= 2
                        t = redpool.tile([128, width], BF16, tag=f"t{width}")
                        nc.vector.tensor_tensor(
                            t, cur[:, 0:width], cur[:, width : 2 * width],
                            mybir.AluOpType.max,
                        )
                        cur = t
                    if width < 2048:
                        # chunk is a fraction of a j-slice: it lands in a
                        # contiguous accP column window [off, off+width)
                        off = (ch * CH_IDX) % 2048
                        dstv = accP[:, off : off + width]
                        if ch * CH_IDX < 2048:
                            nc.vector.tensor_copy(dstv, cur)
                        else:
                            nc.vector.tensor_tensor(
                                dstv, dstv, cur, mybir.AluOpType.max
                            )
                    elif ch == 0:
                        if width == 2048:  # chunk is a single j-slice
                            nc.vector.tensor_copy(accP, cur)
                        else:
                            nc.vector.tensor_tensor(
                                accP, cur[:, 0:2048], cur[:, 2048:4096],
                                mybir.AluOpType.max,
                            )
                    elif width == 2048:
                        nc.vector.tensor_tensor(
                            accP, accP, cur, mybir.AluOpType.max
                        )
                    else:
                        t1 = redpool.tile([128, 2048], BF16, tag="t1")
                        nc.vector.tensor_tensor(
                            t1, cur[:, 0:2048], cur[:, 2048:4096],
                            mybir.AluOpType.max,
                        )
                        nc.vector.tensor_tensor(
                            accP, accP, t1, mybir.AluOpType.max
                        )
                return accP

            def pm_to_fm(accP, tagsfx):
                """wrapped point-major [128, 16*CP] -> feature-major [64, N]."""
                fm = fmpool.tile([C, N], BF16, tag="fm" + tagsfx)
                for q4 in range(4):
                    pt = pst.tile([128, 512], BF16, tag="pt")
                    for qq in range(4):
                        q = q4 * 4 + qq
                        nc.tensor.transpose(
                            pt[:, qq * 128 : (qq + 1) * 128],
                            accP[:, q * CP : (q + 1) * CP],
                            id_sb,
                        )
                    nc.scalar.activation(
                        fm[:, q4 * 512 : (q4 + 1) * 512],
                        pt[:C, :],
                        mybir.ActivationFunctionType.Identity,
                    )
                return fm

            def emit_l1(b):
                """Layer 1 + conv1 for batch b; returns (rt, idxb_sb).

                Emission is phase-split (all emit_l1 calls before any
                emit_l2) so the in-order Pool engine issues both batches'
                L1 gathers back-to-back instead of blocking on batch 0's
                conv1 before batch 1's independent L1 gathers.
                """
                idxa_sb = idxpool.tile([128, NIDX // 16], I16, tag="idxa")
                nc.sync.dma_start(idxa_sb, idxa_d[b])
                idxb_sb = idxpool.tile([128, NIDX // 16], I16, tag="idxb")
                nc.sync.dma_start(idxb_sb, idxb_d[b])

                # ---------- layer 1: gather+max over x ----------
                acc1 = gather_max_layer(xh_d[b], idxa_sb)
                fm1 = pm_to_fm(acc1, "1")

                # ---------- linear1 + bias (feature-major) ----------
                h1 = h1pool.tile([D1, N], BF16, tag="h1")
                for m in range(4):
                    pm = psm.tile([128, 512], F32, tag="pm")
                    nc.tensor.matmul(
                        pm[:D1, :], w1_sb, fm1[:, m * 512 : (m + 1) * 512]
                    )
                    nc.scalar.activation(
                        h1[:, m * 512 : (m + 1) * 512],
                        pm[:D1, :],
                        mybir.ActivationFunctionType.Identity,
                        bias=b1_sb,
                    )

                # ---------- conv1 + bias + relu, point-major output ----------
                rw = rwpool.tile([128, (N // 128) * CP], BF16, tag="rw")
                for q4 in range(4):
                    po = pc1.tile([128, 512], F32, tag="po")
                    nc.tensor.matmul(
                        po, ones_sb[:, :128], bc1r_sb, start=True, stop=False
                    )
                    for qq in range(4):
                        q = q4 * 4 + qq
                        nc.tensor.matmul(
                            po[:, qq * CP : (qq + 1) * CP],
                            h1[:, q * 128 : (q + 1) * 128],
                            wc1t_sb,
                            start=False,
                            stop=(qq == 3),
                        )
                    nc.scalar.activation(
                        rw[:, q4 * 512 : (q4 + 1) * 512],
                        po,
                        mybir.ActivationFunctionType.Relu,
                    )

                # r table to DRAM: row (p*16+q) = point q*128+p (idxb matches)
                rt = drampool.tile([N, CP], BF16, tag="rt")
                nc.sync.dma_start(
                    rt.rearrange("(p q) c -> p (q c)", p=128), rw
                )
                return rt, idxb_sb

            def emit_l2(b, rt, idxb_sb):
                # ---------- layer 2: gather+max over r ----------
                acc2 = gather_max_layer(rt[:], idxb_sb)
                fm2 = pm_to_fm(acc2, "2")

                # ---------- linear2 + bias ----------
                h3 = h3pool.tile([D2, N], BF16, tag="h3")
                for m in range(4):
                    pm = psm.tile([128, 512], F32, tag="pm")
                    nc.tensor.matmul(pm, w2_sb, fm2[:, m * 512 : (m + 1) * 512])
                    nc.scalar.activation(
                        h3[:, m * 512 : (m + 1) * 512],
                        pm,
                        mybir.ActivationFunctionType.Identity,
                        bias=b2_sb,
                    )

                # ---------- conv2 (1024 out channels) + bias ----------
                for dc in range(8):
                    osb = outpool.tile([128, N], F32, tag="osb")
                    for m in range(4):
                        po = pso.tile([128, 512], F32, tag="po2")
                        nc.tensor.matmul(
                            po,
                            bc2r_sb[:, dc * 128 : (dc + 1) * 128],
                            ones_sb,
                            start=True,
                            stop=False,
                        )
                        nc.tensor.matmul(
                            po,
                            wc2t_sb[:, dc * 128 : (dc + 1) * 128],
                            h3[:, m * 512 : (m + 1) * 512],
                            start=False,
                            stop=True,
                        )
                        nc.any.tensor_copy(osb[:, m * 512 : (m + 1) * 512], po)
                    nc.sync.dma_start(out_d[b, dc * 128 : (dc + 1) * 128, :], osb)

            def emit_batches():
                l1 = [emit_l1(b) for b in range(BPC)]
                for b in range(BPC):
                    emit_l2(b, *l1[b])

            if reps == 1:
                emit_batches()
            else:
                with tc.For_i(0, reps, 1):
                    emit_batches()

    nc.compile()
    _verify_gather_queue_alignment(nc, nqueues)
    return nc


def _get_nc():
    global _compiled
    if _compiled is None:
        _compiled = _build_nc()
    return _compiled


def _prep_inputs(x, idx, W1, b1, Wc1, bc1, W2, b2, Wc2, bc2):
    """Host-side sharding + layout marshalling -> per-core in_maps."""
    from ml_dtypes import bfloat16

    x = np.asarray(x, np.float32)
    idx = np.asarray(idx)

    # batch-local indices (reference guarantees idx[b] in [b*N, (b+1)*N))
    local = idx.astype(np.int64) - (np.arange(B, dtype=np.int64) * N)[:, None, None]
    assert local.min() >= 0 and local.max() < N, "idx not batch-local"
    local = local.astype(np.int16)  # (B, N, K)
    # layer-2 table rows are in wrapped order: row (v%128)*16 + v//128 = v
    localb = ((local % 128) * 16 + local // 128).astype(np.int16)

    def wrap(li):
        km = li.transpose(0, 2, 1).reshape(B, NIDX)  # k-major
        w = km.reshape(B, NIDX // 16, 16).transpose(0, 2, 1)  # (B,16,NIDX/16)
        return np.ascontiguousarray(np.tile(w, (1, 8, 1)))  # 128 partitions

    idxa = wrap(local)
    idxb = wrap(localb)

    # x -> [B, N, 128] bf16 rows (64 real features + zero pad)
    xt = x.transpose(0, 2, 1)  # (B, N, C)
    xh = np.zeros((B, N, CP), dtype=bfloat16)
    xh[:, :, :C] = xt.astype(bfloat16)

    wc1t = np.zeros((D1, CP), dtype=bfloat16)
    wc1t[:, :D1] = np.asarray(Wc1, np.float32).T.astype(bfloat16)
    bc1p = np.zeros((CP,), dtype=bfloat16)
    bc1p[:D1] = np.asarray(bc1, np.float32).astype(bfloat16)
    common = {
        "W1": np.ascontiguousarray(np.asarray(W1, np.float32).astype(bfloat16)),
        "Wc1T": wc1t,
        "W2": np.ascontiguousarray(np.asarray(W2, np.float32).astype(bfloat16)),
        "Wc2T": np.ascontiguousarray(np.asarray(Wc2, np.float32).T.astype(bfloat16)),
        "b1": np.asarray(b1, np.float32).reshape(D1, 1),
        "b2": np.asarray(b2, np.float32).reshape(D2, 1),
        "bc1row4": np.ascontiguousarray(np.tile(bc1p, 4).reshape(1, 512)),
        "bc2row": np.asarray(bc2, np.float32).astype(bfloat16).reshape(1, DOUT),
        "ident": np.eye(128, dtype=bfloat16),
    }
    in_maps = []
    for c in range(NCORES):
        bs = [BPC * c + j for j in range(BPC)]
        m = dict(common)
        m["xh"] = np.ascontiguousarray(xh[bs])
        m["idxa"] = np.ascontiguousarray(idxa[bs])
        m["idxb"] = np.ascontiguousarray(idxb[bs])
        in_maps.append(m)
    return in_maps


def kernel(_trace=False, _trace_kwargs=None, **inputs):
    nc = _get_nc()
    in_maps = _prep_inputs(**inputs)
    res = run_bass_kernel_spmd(
        nc,
        in_maps,
        list(range(NCORES)),
        trace=_trace,
        **(_trace_kwargs or {}),
    )
    out = np.empty((B, DOUT, N), np.float32)
    for c in range(NCORES):
        for j in range(BPC):
            out[BPC * c + j] = res.results[c]["out"][j]
    if _trace:
        return out, res
    return out
